# revision 6
# baseline (speedup 1.0000x reference)
"""Trainium2 Bass kernel for nn_MemModule (softmax memory attention + hard
shrink + L1 renorm + readout), data-parallel over 8 NeuronCores.

Computation per row x (FEA=512), memory bank W [MEM=2000, FEA]:
    att_raw = softmax(x @ W.T)                     # [MEM]
    h       = relu(att_raw - LAM) * att_raw / (att_raw - LAM + EPS)
    att     = h / max(sum(h), EPS)
    out     = att @ W                              # [FEA]

kernel(x, weight) -> (out [N,FEA] f32, att [N,MEM] f32)
"""

from contextlib import ExitStack

import numpy as np

import concourse.bass as bass
import concourse.bacc as bacc
import concourse.tile as tile
from concourse import mybir
from concourse.bass_utils import run_bass_kernel_spmd
from concourse.masks import make_identity

P = 128
FEA = 512
MEM = 2000
NCORES = 8
LAM = 0.0025
EPS = 1e-12
FCH = FEA // P                 # 4 fea chunks of 128 (contraction for scores)
MCH = (MEM + P - 1) // P       # 16 mem chunks (last one is 80)
NB = 4                         # psum bank groups for the score row
NBW = MEM // NB                # 500 columns per group

f32 = mybir.dt.float32
f32r = mybir.dt.float32r
AF = mybir.ActivationFunctionType
OP = mybir.AluOpType

_cache = {}


def _build(rows: int):
    nc = bacc.Bacc(
        "TRN2", target_bir_lowering=False, debug=False, num_devices=NCORES
    )
    x_d = nc.dram_tensor("x", [rows, FEA], f32, kind="ExternalInput").ap()
    w_d = nc.dram_tensor("w", [MEM, FEA], f32, kind="ExternalInput").ap()
    wt_d = nc.dram_tensor("wt", [FEA, MEM], f32, kind="ExternalInput").ap()
    att_d = nc.dram_tensor("att", [rows, MEM], f32, kind="ExternalOutput").ap()
    out_d = nc.dram_tensor("out", [rows, FEA], f32, kind="ExternalOutput").ap()
    ntiles = rows // P

    with tile.TileContext(nc) as tc, ExitStack() as ctx:
        consts = ctx.enter_context(tc.tile_pool(name="consts", bufs=1))
        ident = consts.tile([P, P], f32)
        make_identity(nc, ident)
        eps_sb = consts.tile([P, 1], f32)
        nc.vector.memset(eps_sb[:], EPS)
        # W in [mem, fea] layout, chunked along mem for the readout matmul.
        w_sb = consts.tile([P, MCH, FEA], f32)
        for c in range(MCH):
            kc = min(P, MEM - c * P)
            nc.sync.dma_start(out=w_sb[:kc, c, :], in_=w_d[c * P : c * P + kc, :])
        # W.T in [fea, mem] layout, chunked along fea for the scores matmul.
        wt_sb = consts.tile([P, FCH, MEM], f32)
        for k in range(FCH):
            nc.sync.dma_start(out=wt_sb[:, k, :], in_=wt_d[k * P : (k + 1) * P, :])

        xin = ctx.enter_context(tc.tile_pool(name="xin", bufs=3))
        work = ctx.enter_context(tc.tile_pool(name="work", bufs=2))
        outp = ctx.enter_context(tc.tile_pool(name="outp", bufs=3))
        scal = ctx.enter_context(tc.tile_pool(name="scal", bufs=6))
        ps_scores = ctx.enter_context(tc.tile_pool(name="ps_s", bufs=1, space="PSUM"))
        ps_small = ctx.enter_context(tc.tile_pool(name="ps_t", bufs=3, space="PSUM"))
        ps_out = ctx.enter_context(tc.tile_pool(name="ps_o", bufs=1, space="PSUM"))

        for it in range(ntiles):
            r0 = it * P
            x_t = xin.tile([P, FEA], f32, tag="x")
            nc.sync.dma_start(out=x_t[:], in_=x_d[r0 : r0 + P, :])

            # xT chunks [fea_k, rows] via PE transpose (lhsT for scores).
            xt = xin.tile([P, FCH, P], f32, tag="xt")
            for k in range(FCH):
                pt = ps_small.tile([P, P], f32, tag="pt")
                nc.tensor.transpose(pt[:], x_t[:, k * P : (k + 1) * P], ident[:])
                nc.scalar.copy(xt[:, k, :], pt[:])

            # scores[rows, mem] = x @ W.T, fp32r, into 4 psum banks.
            ps = ps_scores.tile([P, NB, 512], f32)
            for k in range(FCH):
                for j in range(NB):
                    nc.tensor.matmul(
                        ps[:, j, :NBW],
                        xt[:, k, :],
                        wt_sb[:, k, j * NBW : (j + 1) * NBW],
                        start=(k == 0),
                        stop=(k == FCH - 1),
                    )

            # expt = exp(scores) (no max-subtraction: logits are in [-4, 4]),
            # fused per-row sum via accum_out.
            expt = work.tile([P, MEM], f32, tag="expt")
            sumexp = scal.tile([P, 1], f32, tag="sumexp")
            nc.scalar.activation(
                expt[:].rearrange("p (c n) -> p c n", c=NB),
                ps[:, :, :NBW],
                AF.Exp,
                accum_out=sumexp[:],
            )
            r1 = scal.tile([P, 1], f32, tag="r1")
            nc.vector.reciprocal(r1[:], sumexp[:])

            # t = att_raw - LAM = expt*r1 - LAM   (att_raw = softmax row)
            t_t = work.tile([P, MEM], f32, tag="t")
            nc.vector.tensor_scalar(t_t[:], expt[:], r1[:], LAM, OP.mult, OP.subtract)
            # m = relu(t)
            m_t = work.tile([P, MEM], f32, tag="m")
            nc.scalar.activation(m_t[:], t_t[:], AF.Relu)
            # d = t + EPS (in place over t), rd ~= 1/d (51-ulp approx, in place)
            nc.scalar.activation(t_t[:], t_t[:], AF.Identity, bias=eps_sb[:])
            nc.vector.reciprocal_approx_fast(t_t[:], t_t[:])
            # q = m * rd (in place over m); u = q * expt (in place over expt)
            nc.vector.tensor_tensor(m_t[:], m_t[:], t_t[:], OP.mult)
            nc.vector.tensor_tensor(expt[:], m_t[:], expt[:], OP.mult)
            # h = u * r1 (in place over expt), fused row-sum of h
            sum_h = scal.tile([P, 1], f32, tag="sum_h")
            nc.vector.tensor_scalar(
                expt[:], expt[:], r1[:], 0.0, OP.mult, OP.add, accum_out=sum_h[:]
            )
            # r2 = 1 / max(sum_h, EPS)
            r2 = scal.tile([P, 1], f32, tag="r2")
            nc.vector.tensor_scalar_max(r2[:], sum_h[:], EPS)
            nc.vector.reciprocal(r2[:], r2[:])

            # att = h * r2 -> DMA out
            att_t = work.tile([P, MEM], f32, tag="att")
            nc.vector.tensor_scalar(att_t[:], expt[:], r2[:], None, OP.mult)
            nc.sync.dma_start(out=att_d[r0 : r0 + P, :], in_=att_t[:])

            # hT chunks [mem_c, rows] via PE transpose (lhsT for readout).
            ht = work.tile([P, MCH, P], f32, tag="ht")
            for c in range(MCH):
                kc = min(P, MEM - c * P)
                pt = ps_small.tile([P, P], f32, tag="pt")
                nc.tensor.transpose(
                    pt[:kc, :], expt[:, c * P : c * P + kc], ident[:]
                )
                nc.scalar.copy(ht[:kc, c, :], pt[:kc, :])

            # out = (h @ W) * r2, accumulated over 16 mem chunks.
            po = ps_out.tile([P, FEA], f32)
            for c in range(MCH):
                kc = min(P, MEM - c * P)
                nc.tensor.matmul(
                    po[:],
                    ht[:kc, c, :],
                    w_sb[:kc, c, :],
                    start=(c == 0),
                    stop=(c == MCH - 1),
                )
            out_t = outp.tile([P, FEA], f32, tag="out")
            nc.vector.tensor_scalar(out_t[:], po[:], r2[:], None, OP.mult)
            nc.sync.dma_start(out=out_d[r0 : r0 + P, :], in_=out_t[:])

    nc.compile()
    return nc


def _get_nc(rows: int):
    if rows not in _cache:
        _cache[rows] = _build(rows)
    return _cache[rows]


def run(x, weight, trace=False):
    """Returns ((out, att), BassKernelResults)."""
    x = np.ascontiguousarray(x, dtype=np.float32)
    weight = np.ascontiguousarray(weight, dtype=np.float32)
    n = x.shape[0]
    assert n % (NCORES * P) == 0, n
    rows = n // NCORES
    nc = _get_nc(rows)
    wt = np.ascontiguousarray(weight.T)
    in_maps = [
        {"x": x[i * rows : (i + 1) * rows], "w": weight, "wt": wt}
        for i in range(NCORES)
    ]
    res = run_bass_kernel_spmd(
        nc, in_maps, core_ids=list(range(NCORES)), trace=trace
    )
    out = np.concatenate([r["out"] for r in res.results], axis=0)
    att = np.concatenate([r["att"] for r in res.results], axis=0)
    return (out, att), res


def kernel(x, weight):
    (out, att), _ = run(x, weight)
    return out, att
